# revision 1
# baseline (speedup 1.0000x reference)
import sys

sys.path.insert(0, "/opt/trn_rl_repo")

import numpy as np
import ml_dtypes

import concourse.mybir as mybir
from concourse import bass, tile
from concourse import tile_sem_assignment as _tsa
from concourse.bass_utils import run_bass_kernel_spmd
from concourse.vector_clock import ScopedClock, VectorClock

_orig_drain_and_barrier = tile.TileContext._drain_and_barrier


def _split_drain_and_barrier(self, tick_clock, wait_clock):
    # The final Drain waits on every active semaphore at once; with 8 HWDGE
    # lanes + SWDGE + 3 engines that exceeds the CTRL instruction's sync
    # wait slots. Emit one 1-wait drain per proc instead (same semantics:
    # SP executes them in order, so all sems reach their targets before the
    # barrier), then replicate the original barrier/cleanup sequence.
    gc = tick_clock.global_clock
    n = _tsa.N_PROCS
    for p in range(n):
        if gc[p] > 0:
            partial = VectorClock([gc[q] if q == p else 0 for q in range(n)])
            d = self.nc.sync.drain()
            wait_clock.add_sem_waits(d.ins, ScopedClock({None: partial}))
    self.nc.all_engine_barrier()
    popped = self.nc._tile_sem_poison_stack.pop()
    assert popped is self._sem_poison
    self.nc.clear_and_free_semaphores(list(self.sems.allocated().values()))
    self.nc.all_engine_barrier()


tile.TileContext._drain_and_barrier = _split_drain_and_barrier

B = 1024        # batch rows of address
N = 65536       # mem rows (sharded)
M = 128         # mem cols
NCORES = 8
NS = N // NCORES          # 8192 rows per core
NCHUNKS = NS // 128       # 64 chunks of 128 mem-rows
MCHUNKS = NS // 256       # 32 mega-chunks of 256 mem-rows (DoubleRow)
BCHUNKS = B // 128        # 8 chunks of 128 batch-rows
NSTAGES = 8               # DMA pipeline stages (8 chunks each)

FP8 = mybir.dt.float8e4
BF16 = mybir.dt.bfloat16
F32 = mybir.dt.float32
DR = mybir.MatmulPerfMode.DoubleRow
ADD = mybir.AluOpType.add
MULT = mybir.AluOpType.mult

_compiled = {}


NG = 16  # DMA groups; each covers 4 mem-chunks (k) = 2 mega-chunks (ch)


def _build_nc():
    nc = bass.Bass(target_bir_lowering=False)

    # a:  [p=b%128, j(n-slice of 1024), ub(u-block), bc, u]  A shard for GEMM1
    a = nc.dram_tensor("a", [128, NSTAGES, 8, BCHUNKS, 128], FP8, kind="ExternalInput")
    # at: [p=n%128 within 256-chunk, ch, sub, b]  A^T shard for GEMM2 (partition=n)
    at = nc.dram_tensor("at", [128, MCHUNKS, 2, B], FP8, kind="ExternalInput")
    # c:  [p=n%128, k, m]  0.5*content shard (partition=n)
    c = nc.dram_tensor("c", [128, NCHUNKS, M], FP8, kind="ExternalInput")
    # ed: [p=b%128, h(hi/lo), bc, 2M]  [-erase | 0.5*add] split as hi+lo fp8
    ed = nc.dram_tensor("ed", [128, 2, BCHUNKS, 2 * M], FP8, kind="ExternalInput")
    # rt: [m, b] partial (read/2)^T fp32
    rt = nc.dram_tensor("rt", [M, B], F32, kind="ExternalOutput")

    with tile.TileContext(nc) as tc:
        with (
            tc.tile_pool(name="abuf", bufs=1) as a_pool,
            tc.tile_pool(name="atbuf", bufs=1) as at_pool,
            tc.tile_pool(name="cbuf", bufs=1) as c_pool,
            tc.tile_pool(name="edbuf", bufs=1) as ed_pool,
            tc.tile_pool(name="tmpbuf", bufs=8) as tmp_pool,
            tc.tile_pool(name="cpbuf", bufs=6) as cp_pool,
            tc.tile_pool(name="rtbuf", bufs=1) as rt_pool,
            tc.tile_pool(name="pw", bufs=6, space="PSUM") as pw_pool,
            tc.tile_pool(name="pr", bufs=1, space="PSUM") as pr_pool,
        ):
            a_t = a_pool.tile([128, NSTAGES, 8, BCHUNKS, 128], FP8)
            at_t = at_pool.tile([128, MCHUNKS, 2, B], FP8)
            c_t = c_pool.tile([128, NCHUNKS, M], FP8)
            ed_t = ed_pool.tile([128, 2, BCHUNKS, 2 * M], FP8)

            # Fine-grained preloads in consumption order so DMA arrival
            # tracks compute need and the scheduler interleaves G1/G2
            # naturally (the wait-dedup chain needs G2 close behind G1).
            # Preload DMAs write each SBUF dest exactly once, so their only
            # wait is the HWDGE lane-credit wait (1 wait, allowed). The rt
            # store goes out over SWDGE (gpsimd) so it lands on a fresh
            # lane and carries only its RAW wait.
            def a_group(g):
                j, ub0 = g // 2, (g % 2) * 4
                nc.sync.dma_start(
                    out=a_t[:, j, ub0 : ub0 + 4], in_=a[:, j, ub0 : ub0 + 4]
                )

            # 'a' leads 'at' by one group so the G1->STT->TADD chain for the
            # final chunks drains while the last at groups are still in
            # flight; the last at group is split per mega-chunk so only
            # G2(31) + copy/store trail the final DMA.
            nc.sync.dma_start(out=ed_t[:], in_=ed[:])
            nc.sync.dma_start(out=c_t[:, 0:32, :], in_=c[:, 0:32, :])
            a_group(0)
            for g in range(NG - 2):
                a_group(g + 1)
                if g == 7:
                    nc.sync.dma_start(out=c_t[:, 32:64, :], in_=c[:, 32:64, :])
                nc.sync.dma_start(
                    out=at_t[:, 2 * g : 2 * g + 2], in_=at[:, 2 * g : 2 * g + 2]
                )
            a_group(NG - 1)
            nc.sync.dma_start(out=at_t[:, 28:30], in_=at[:, 28:30])
            nc.sync.dma_start(out=at_t[:, 30:31], in_=at[:, 30:31])
            nc.sync.dma_start(out=at_t[:, 31:32], in_=at[:, 31:32])

            psum_r = pr_pool.tile([128, B], F32)
            land = tmp_pool.tile([128, 1], F32)
            # Wake the Activation engine early: its first instruction carries
            # a ~1.4us startup cost in the model; pay it off the critical
            # path so the tail copies run at steady-state rate.
            warm = tmp_pool.tile([128, 1], F32)
            nc.scalar.copy(warm[:], ed_t[:, 0, 0, 0:1])

            def emit_g2(ch, cp):
                for jj in range(2):
                    nc.tensor.matmul(
                        psum_r[:, jj * 512 : (jj + 1) * 512],
                        cp[:],
                        at_t[:, ch, :, jj * 512 : (jj + 1) * 512],
                        start=(ch == 0),
                        stop=(ch == MCHUNKS - 1),
                        perf_mode=DR,
                    )

            cp = None
            for k in range(NCHUNKS):
                ch, sub = k // 2, k % 2
                j, ub = k // 8, k % 8
                if k % 32 == 0:
                    # DVE absorbs this c-half's DMA wait so STT(k) keeps
                    # only its PSUM-read wait (dedup on the same lane sem).
                    nc.vector.tensor_copy(land[:], c_t[:, k, 0:1])

                if sub == 0:
                    cp = cp_pool.tile([128, 2, M], FP8)

                psum_w = pw_pool.tile([128, 2 * M], F32)
                for h in range(2):
                    for q in range(4):
                        nc.tensor.matmul(
                            psum_w[:],
                            a_t[:, j, ub, 2 * q : 2 * q + 2, :],
                            ed_t[:, h, 2 * q : 2 * q + 2, :],
                            start=(h == 0 and q == 0),
                            stop=(h == 1 and q == 3),
                            perf_mode=DR,
                        )

                # psum_w = [-We | Wa/2];  C'/2 = (1 - We) * (C/2) + Wa/2
                tmp2 = tmp_pool.tile([128, M], F32)
                nc.vector.scalar_tensor_tensor(
                    tmp2[:], psum_w[:, 0:M], 1.0, c_t[:, k, :], ADD, MULT
                )
                nc.vector.tensor_add(cp[:, sub, :], tmp2[:], psum_w[:, M : 2 * M])

                if sub == 1:
                    # G2 Ldweights (stationary=cp) carries DVE>=tadd(2ch+1),
                    # covering the bank-WAR waits of later G1 start-matmuls
                    # via per-engine wait dedup.
                    emit_g2(ch, cp)

            # Split the tail: psum_r bank jj completes at G2(ch=31, jj), so
            # copy+store each half as soon as its accumulation stops instead
            # of one serial full-width copy followed by one big store.
            rt_t = rt_pool.tile([128, B], F32)
            for jj in range(2):
                nc.scalar.copy(
                    rt_t[:, jj * 512 : (jj + 1) * 512],
                    psum_r[:, jj * 512 : (jj + 1) * 512],
                )
                nc.gpsimd.dma_start(
                    out=rt[:, jj * 512 : (jj + 1) * 512],
                    in_=rt_t[:, jj * 512 : (jj + 1) * 512],
                )

    # The scheduler can hoist a G1 start-Matmult ahead of the G2 Ldweights
    # whose DVE wait would dedup-cover its bank-WAR wait, leaving it with
    # two waits (PE self-wait + DVE) — one over the HW wait-slot limit.
    # The same-engine self-wait is always satisfied by in-order queue
    # completion, so drop it.
    for inst in nc.inst_map.values():
        si = inst.sync_info
        if si and si.on_wait and len(si.on_wait) > 1:
            eng = str(inst.engine).split(".")[-1]
            kept = [w for w in si.on_wait if not w.ant_name.startswith(eng + "_")]
            assert len(kept) == 1
            si.on_wait = kept

    return nc


def _prep_inputs(address, erase, add, content):
    f8 = ml_dtypes.float8_e4m3
    a_f8 = address.astype(f8)                                 # [1024, 65536]
    ed = np.concatenate([-erase, 0.5 * add], axis=1)          # [1024, 256] f32
    ed_hi = ed.astype(f8)
    ed_lo = (ed - ed_hi.astype(np.float32)).astype(f8)
    ed_st = np.stack([ed_hi, ed_lo])                          # [2, 1024, 256]
    ed_r = np.ascontiguousarray(
        ed_st.reshape(2, BCHUNKS, 128, 2 * M).transpose(2, 0, 1, 3)
    )                                                         # [128, 2, 8, 256]
    c_bf = (0.5 * content).astype(f8)                         # [65536, 128]

    in_maps = []
    for ci in range(NCORES):
        a_c = a_f8[:, ci * NS : (ci + 1) * NS]                # [1024, 8192]
        # a_r[p, j, ub, bc, u] = A[bc*128+p, j*1024+ub*128+u]
        a_r = np.ascontiguousarray(
            a_c.reshape(BCHUNKS, 128, NSTAGES, 8, 128).transpose(1, 2, 3, 0, 4)
        )                                                     # [128, 8, 8, 8, 128]
        # at_r[p, ch, s, b] = A[b, ch*256 + s*128 + p]
        at_r = np.ascontiguousarray(
            a_c.T.reshape(MCHUNKS, 2, 128, B).transpose(2, 0, 1, 3)
        )                                                     # [128, 32, 2, 1024]
        c_c = c_bf[ci * NS : (ci + 1) * NS, :]
        c_r = np.ascontiguousarray(
            c_c.reshape(NCHUNKS, 128, M).transpose(1, 0, 2)
        )                                                     # [128, 64, 128]
        in_maps.append({"a": a_r, "at": at_r, "c": c_r, "ed": ed_r})
    return in_maps


def kernel(address, erase, add, content, _trace=False, _result_box=None):
    if "nc" not in _compiled:
        _compiled["nc"] = _build_nc()
    nc = _compiled["nc"]

    in_maps = _prep_inputs(address, erase, add, content)
    res = run_bass_kernel_spmd(
        nc, in_maps, core_ids=list(range(NCORES)), trace=_trace
    )
    if _result_box is not None:
        _result_box.append(res)

    acc = np.zeros((M, B), dtype=np.float32)
    for r in res.results:
        acc += np.asarray(r["rt"], dtype=np.float32)
    return np.ascontiguousarray((2.0 * acc).T)



# revision 5
# speedup vs baseline: 1.4873x; 1.4873x over previous
import sys

sys.path.insert(0, "/opt/trn_rl_repo")

import numpy as np
import ml_dtypes

import concourse.mybir as mybir
from concourse import bass, tile
from concourse import tile_sem_assignment as _tsa
from concourse.bass_utils import run_bass_kernel_spmd
from concourse.vector_clock import ScopedClock, VectorClock

_orig_drain_and_barrier = tile.TileContext._drain_and_barrier


def _split_drain_and_barrier(self, tick_clock, wait_clock):
    # The final Drain waits on every active semaphore at once; with 8 HWDGE
    # lanes + SWDGE + 3 engines that exceeds the CTRL instruction's sync
    # wait slots. Emit one 1-wait drain per proc instead (same semantics:
    # SP executes them in order, so all sems reach their targets before the
    # barrier), then replicate the original barrier/cleanup sequence.
    gc = tick_clock.global_clock
    n = _tsa.N_PROCS
    for p in range(n):
        if gc[p] > 0:
            partial = VectorClock([gc[q] if q == p else 0 for q in range(n)])
            d = self.nc.sync.drain()
            wait_clock.add_sem_waits(d.ins, ScopedClock({None: partial}))
    self.nc.all_engine_barrier()
    popped = self.nc._tile_sem_poison_stack.pop()
    assert popped is self._sem_poison
    self.nc.clear_and_free_semaphores(list(self.sems.allocated().values()))
    self.nc.all_engine_barrier()


tile.TileContext._drain_and_barrier = _split_drain_and_barrier

B = 1024        # batch rows of address
N = 65536       # mem rows (sharded)
M = 128         # mem cols
NCORES = 8
NS = N // NCORES          # 8192 rows per core
NCHUNKS = NS // 128       # 64 chunks of 128 mem-rows
MCHUNKS = NS // 256       # 32 mega-chunks of 256 mem-rows (DoubleRow)
BCHUNKS = B // 128        # 8 chunks of 128 batch-rows
NSTAGES = 8               # DMA pipeline stages (8 chunks each)

FP8 = mybir.dt.float8e4
BF16 = mybir.dt.bfloat16
F32 = mybir.dt.float32
DR = mybir.MatmulPerfMode.DoubleRow
ADD = mybir.AluOpType.add
MULT = mybir.AluOpType.mult

_compiled = {}


NG = 16  # a DMA groups; each covers 4 mem-chunks (k) = 2 mega-chunks (ch)


def _build_nc():
    nc = bass.Bass(target_bir_lowering=False)

    # a:  [p=b%128, j(n-slice of 1024), ub(u-block), bc, u]  A shard for GEMM1
    a = nc.dram_tensor("a", [128, NSTAGES, 8, BCHUNKS, 128], FP8, kind="ExternalInput")
    # at: [p=n%128 within 256-chunk, ch, sub, b]  A^T shard for GEMM2 (partition=n)
    at = nc.dram_tensor("at", [128, MCHUNKS, 2, B], FP8, kind="ExternalInput")
    # c:  [p=n%128, k, m]  0.5*content shard (partition=n)
    c = nc.dram_tensor("c", [128, NCHUNKS, M], FP8, kind="ExternalInput")
    # ed: [p=b%128, bc, 2M]  [-erase | 0.5*add] fp8 (hi only)
    ed = nc.dram_tensor("ed", [128, BCHUNKS, 2 * M], FP8, kind="ExternalInput")
    # rt: [m, b] partial (read/2)^T bf16
    rt = nc.dram_tensor("rt", [M, B], BF16, kind="ExternalOutput")

    with tile.TileContext(nc) as tc:
        with (
            tc.tile_pool(name="abuf", bufs=1) as a_pool,
            tc.tile_pool(name="atbuf", bufs=1) as at_pool,
            tc.tile_pool(name="cbuf", bufs=1) as c_pool,
            tc.tile_pool(name="edbuf", bufs=1) as ed_pool,
            tc.tile_pool(name="tmpbuf", bufs=8) as tmp_pool,
            tc.tile_pool(name="cpbuf", bufs=6) as cp_pool,
            tc.tile_pool(name="rtbuf", bufs=1) as rt_pool,
            tc.tile_pool(name="pw", bufs=6, space="PSUM") as pw_pool,
            tc.tile_pool(name="pr", bufs=1, space="PSUM") as pr_pool,
        ):
            a_t = a_pool.tile([128, NSTAGES, 8, BCHUNKS, 128], FP8)
            at_t = at_pool.tile([128, MCHUNKS, 2, B], FP8)
            c_t = c_pool.tile([128, NCHUNKS, M], FP8)
            ed_t = ed_pool.tile([128, BCHUNKS, 2 * M], FP8)

            # Input DMAs split across the available queues so the shared DMA
            # device (the real serial resource) is never starved by one
            # queue's per-instruction issue overhead: 'a' on SP, 'at' on
            # Activation (the two HWDGE queues), ed+c on Pool (SWDGE).
            # Preload DMAs write each SBUF dest exactly once, so their only
            # wait is the HWDGE lane-credit wait.
            def a_group(g):
                j, ub0 = g // 2, (g % 2) * 4
                nc.sync.dma_start(
                    out=a_t[:, j, ub0 : ub0 + 4], in_=a[:, j, ub0 : ub0 + 4]
                )

            nc.gpsimd.dma_start(out=ed_t[:], in_=ed[:])
            for g in range(NG):
                a_group(g)
            nc.gpsimd.dma_start(out=c_t[:, 0:32, :], in_=c[:, 0:32, :])
            nc.gpsimd.dma_start(out=c_t[:, 32:64, :], in_=c[:, 32:64, :])
            # at stream: pairs of mega-chunks; tail split fine so only the
            # last G2 + copy/store trail the final (small) transfer.
            for g in range(14):
                nc.scalar.dma_start(
                    out=at_t[:, 2 * g : 2 * g + 2], in_=at[:, 2 * g : 2 * g + 2]
                )
            nc.scalar.dma_start(out=at_t[:, 28:30], in_=at[:, 28:30])
            nc.scalar.dma_start(out=at_t[:, 30:31], in_=at[:, 30:31])
            nc.scalar.dma_start(
                out=at_t[:, 31:32, :, 0:512], in_=at[:, 31:32, :, 0:512]
            )
            nc.scalar.dma_start(
                out=at_t[:, 31:32, :, 512:1024], in_=at[:, 31:32, :, 512:1024]
            )

            psum_r = pr_pool.tile([128, B], F32)
            land = tmp_pool.tile([128, 1], F32)
            # Wake the Activation engine early: its first instruction carries
            # a ~1.3us act-table load in the model; pay it off the critical
            # path so the tail copies run at steady-state rate. Source tile
            # is memset by Pool (idle) to avoid waiting on any DMA.
            wsrc = tmp_pool.tile([128, 1], F32)
            warm = tmp_pool.tile([128, 1], F32)
            nc.gpsimd.memset(wsrc[:], 0.0)
            nc.scalar.copy(warm[:], wsrc[:])

            def emit_g2(ch, cp):
                for jj in range(2):
                    nc.tensor.matmul(
                        psum_r[:, jj * 512 : (jj + 1) * 512],
                        cp[:],
                        at_t[:, ch, :, jj * 512 : (jj + 1) * 512],
                        start=(ch == 0),
                        stop=(ch == MCHUNKS - 1),
                        perf_mode=DR,
                    )

            cp = None
            for k in range(NCHUNKS):
                ch, sub = k // 2, k % 2
                j, ub = k // 8, k % 8
                if k % 32 == 0:
                    # DVE absorbs this c-half's DMA wait so STT(k) keeps
                    # only its PSUM-read wait (dedup on the same lane sem).
                    nc.vector.tensor_copy(land[:], c_t[:, k, 0:1])

                if sub == 0:
                    cp = cp_pool.tile([128, 2, M], FP8)

                psum_w = pw_pool.tile([128, 2 * M], F32)
                for q in range(4):
                    nc.tensor.matmul(
                        psum_w[:],
                        a_t[:, j, ub, 2 * q : 2 * q + 2, :],
                        ed_t[:, 2 * q : 2 * q + 2, :],
                        start=(q == 0),
                        stop=(q == 3),
                        perf_mode=DR,
                    )

                # psum_w = [-We | Wa/2];  C'/2 = (1 - We) * (C/2) + Wa/2
                tmp2 = tmp_pool.tile([128, M], F32)
                nc.vector.scalar_tensor_tensor(
                    tmp2[:], psum_w[:, 0:M], 1.0, c_t[:, k, :], ADD, MULT
                )
                nc.vector.tensor_add(cp[:, sub, :], tmp2[:], psum_w[:, M : 2 * M])

                if sub == 1:
                    # G2 Ldweights (stationary=cp) carries DVE>=tadd(2ch+1),
                    # covering the bank-WAR waits of later G1 start-matmuls
                    # via per-engine wait dedup.
                    emit_g2(ch, cp)

            # Split the tail: psum_r bank jj completes at G2(ch=31, jj), so
            # copy+store each half as soon as its accumulation stops. Copies
            # on Act (warmed), stores spread over Pool then SP so neither
            # store queues behind the other's issue overhead.
            rt_t = rt_pool.tile([128, B], BF16)
            nc.scalar.copy(rt_t[:, 0:512], psum_r[:, 0:512])
            s0 = nc.gpsimd.dma_start(out=rt[:, 0:512], in_=rt_t[:, 0:512])
            nc.scalar.copy(rt_t[:, 512:1024], psum_r[:, 512:1024])
            s1 = nc.sync.dma_start(out=rt[:, 512:1024], in_=rt_t[:, 512:1024])
            store_names = {s0.ins.name, s1.ins.name}

    # The scheduler can hoist a G1 start-Matmult ahead of the G2 Ldweights
    # whose DVE wait would dedup-cover its bank-WAR wait, leaving it with
    # two waits (PE self-wait + DVE) — one over the HW wait-slot limit.
    # The same-engine self-wait is always satisfied by in-order queue
    # completion, so drop it.
    # The rt stores' RAW wait (on the tail Act copy) transitively follows
    # every input DMA completing, so a DMA-lane credit wait on them is
    # always already satisfied — drop it to stay within the 1-wait limit.
    for inst in nc.inst_map.values():
        si = inst.sync_info
        if si and si.on_wait and len(si.on_wait) > 1:
            eng = str(inst.engine).split(".")[-1]
            kept = [w for w in si.on_wait if not w.ant_name.startswith(eng + "_")]
            if len(kept) > 1 and inst.name in store_names:
                kept = [w for w in kept if not w.ant_name.startswith("DMA")]
            assert len(kept) == 1, (inst.name, [w.ant_name for w in si.on_wait])
            si.on_wait = kept

    return nc


def _prep_inputs(address, erase, add, content):
    f8 = ml_dtypes.float8_e4m3
    a_f8 = address.astype(f8)                                 # [1024, 65536]
    ed = np.concatenate([-erase, 0.5 * add], axis=1)          # [1024, 256] f32
    ed_r = np.ascontiguousarray(
        ed.astype(f8).reshape(BCHUNKS, 128, 2 * M).transpose(1, 0, 2)
    )                                                         # [128, 8, 256]
    c_f8 = (0.5 * content).astype(f8)                         # [65536, 128]

    in_maps = []
    for ci in range(NCORES):
        a_c = a_f8[:, ci * NS : (ci + 1) * NS]                # [1024, 8192]
        # a_r[p, j, ub, bc, u] = A[bc*128+p, j*1024+ub*128+u]
        a_r = np.ascontiguousarray(
            a_c.reshape(BCHUNKS, 128, NSTAGES, 8, 128).transpose(1, 2, 3, 0, 4)
        )                                                     # [128, 8, 8, 8, 128]
        # at_r[p, ch, s, b] = A[b, ch*256 + s*128 + p]
        at_r = np.ascontiguousarray(
            a_c.T.reshape(MCHUNKS, 2, 128, B).transpose(2, 0, 1, 3)
        )                                                     # [128, 32, 2, 1024]
        c_c = c_f8[ci * NS : (ci + 1) * NS, :]
        c_r = np.ascontiguousarray(
            c_c.reshape(NCHUNKS, 128, M).transpose(1, 0, 2)
        )                                                     # [128, 64, 128]
        in_maps.append({"a": a_r, "at": at_r, "c": c_r, "ed": ed_r})
    return in_maps


def kernel(address, erase, add, content, _trace=False, _result_box=None):
    if "nc" not in _compiled:
        _compiled["nc"] = _build_nc()
    nc = _compiled["nc"]

    in_maps = _prep_inputs(address, erase, add, content)
    res = run_bass_kernel_spmd(
        nc, in_maps, core_ids=list(range(NCORES)), trace=_trace
    )
    if _result_box is not None:
        _result_box.append(res)

    acc = np.zeros((M, B), dtype=np.float32)
    for r in res.results:
        acc += np.asarray(r["rt"], dtype=np.float32)
    return np.ascontiguousarray((2.0 * acc).T)


# revision 12
# speedup vs baseline: 1.8904x; 1.2710x over previous
import sys

sys.path.insert(0, "/opt/trn_rl_repo")

import numpy as np
import ml_dtypes

import concourse.mybir as mybir
from concourse import bass, tile
from concourse import tile_sem_assignment as _tsa
from concourse.bass_utils import run_bass_kernel_spmd
from concourse.vector_clock import ScopedClock, VectorClock

_orig_drain_and_barrier = tile.TileContext._drain_and_barrier


def _split_drain_and_barrier(self, tick_clock, wait_clock):
    # The final Drain waits on every active semaphore at once; with 8 HWDGE
    # lanes + SWDGE + 3 engines that exceeds the CTRL instruction's sync
    # wait slots. Emit one 1-wait drain per proc instead (same semantics:
    # SP executes them in order, so all sems reach their targets before the
    # barrier), then replicate the original barrier/cleanup sequence.
    gc = tick_clock.global_clock
    n = _tsa.N_PROCS
    for p in range(n):
        if gc[p] > 0:
            partial = VectorClock([gc[q] if q == p else 0 for q in range(n)])
            d = self.nc.sync.drain()
            wait_clock.add_sem_waits(d.ins, ScopedClock({None: partial}))
    self.nc.all_engine_barrier()
    popped = self.nc._tile_sem_poison_stack.pop()
    assert popped is self._sem_poison
    self.nc.clear_and_free_semaphores(list(self.sems.allocated().values()))
    self.nc.all_engine_barrier()


tile.TileContext._drain_and_barrier = _split_drain_and_barrier

B = 1024        # batch rows of address
N = 65536       # mem rows (sharded)
M = 128         # mem cols
NCORES = 8
NS = N // NCORES          # 8192 rows per core
NCHUNKS = NS // 128       # 64 chunks of 128 mem-rows
MCHUNKS = NS // 256       # 32 mega-chunks of 256 mem-rows (DoubleRow)
BCHUNKS = B // 128        # 8 chunks of 128 batch-rows
NSTAGES = 8               # DMA pipeline stages (8 chunks each)

FP8 = mybir.dt.float8e4
BF16 = mybir.dt.bfloat16
F32 = mybir.dt.float32
DR = mybir.MatmulPerfMode.DoubleRow
ADD = mybir.AluOpType.add
MULT = mybir.AluOpType.mult

_compiled = {}


NG = 16  # a DMA groups; each covers 4 mem-chunks (k) = 2 mega-chunks (ch)


def _build_nc():
    nc = bass.Bass(target_bir_lowering=False)

    # a:  [p=b%128, j(n-slice of 1024), ub(u-block), bc, u]  A shard for GEMM1
    a = nc.dram_tensor("a", [128, NSTAGES, 8, BCHUNKS, 128], FP8, kind="ExternalInput")
    # at: [p=n%128 within 256-chunk, ch, sub, b]  A^T shard for GEMM2 (partition=n)
    at = nc.dram_tensor("at", [128, MCHUNKS, 2, B], FP8, kind="ExternalInput")
    # c:  [p=n%128, k, m]  0.5*content shard (partition=n)
    c = nc.dram_tensor("c", [128, NCHUNKS, M], FP8, kind="ExternalInput")
    # ed: [p=b%128, bc, 2M]  [-erase | 0.5*add] fp8 (hi only)
    ed = nc.dram_tensor("ed", [128, BCHUNKS, 2 * M], FP8, kind="ExternalInput")
    # rt: [m, b] partial (read/2)^T bf16
    rt = nc.dram_tensor("rt", [M, B], BF16, kind="ExternalOutput")

    with tile.TileContext(nc) as tc:
        with (
            tc.tile_pool(name="abuf", bufs=1) as a_pool,
            tc.tile_pool(name="atbuf", bufs=1) as at_pool,
            tc.tile_pool(name="cbuf", bufs=1) as c_pool,
            tc.tile_pool(name="edbuf", bufs=1) as ed_pool,
            tc.tile_pool(name="tmpbuf", bufs=8) as tmp_pool,
            tc.tile_pool(name="cpbuf", bufs=6) as cp_pool,
            tc.tile_pool(name="rtbuf", bufs=2) as rt_pool,
            tc.tile_pool(name="pw", bufs=3, space="PSUM") as pw_pool,
            tc.tile_pool(name="pr", bufs=1, space="PSUM") as pr_pool,
        ):
            a_t = a_pool.tile([128, NSTAGES, 8, BCHUNKS, 128], FP8)
            at_t = at_pool.tile([128, MCHUNKS, 2, B], FP8)
            c_t = c_pool.tile([128, NCHUNKS, M], FP8)
            ed_t = ed_pool.tile([128, BCHUNKS, 2 * M], FP8)

            # Transfers issued from different queues run concurrently in the
            # model (the engine SEQ is the serial resource, ~1.58us per
            # 512KB DMA), so spread the 36 input DMAs across all three DMA
            # queues (SP + Act HWDGE, Pool SWDGE) at ~equal cost, ordered
            # within each queue to track compute's consumption order.
            # Preload DMAs write each SBUF dest exactly once, so their only
            # wait is the DGE lane-credit wait.
            def a_group(q, g):
                j, ub0 = g // 2, (g % 2) * 4
                q.dma_start(out=a_t[:, j, ub0 : ub0 + 4], in_=a[:, j, ub0 : ub0 + 4])

            def at_pair(q, p):
                q.dma_start(out=at_t[:, 2 * p : 2 * p + 2], in_=at[:, 2 * p : 2 * p + 2])

            # SP: the first 13 a-groups (chunks 0..51).
            for g in range(13):
                a_group(nc.sync, g)
            # Act: at pairs 0..9 (ch 0..19), then the fine-grained tail
            # pieces so only the last G2 + copy/store trail the final
            # (small) transfer.
            for p in range(10):
                at_pair(nc.scalar, p)
            nc.scalar.dma_start(out=at_t[:, 30:31], in_=at[:, 30:31])
            nc.scalar.dma_start(
                out=at_t[:, 31:32, :, 0:512], in_=at[:, 31:32, :, 0:512]
            )
            nc.scalar.dma_start(
                out=at_t[:, 31:32, :, 512:1024], in_=at[:, 31:32, :, 512:1024]
            )
            # Pool: ed + c (needed first), remaining at pairs, last a-groups.
            nc.gpsimd.dma_start(out=ed_t[:], in_=ed[:])
            nc.gpsimd.dma_start(out=c_t[:, 0:32, :], in_=c[:, 0:32, :])
            at_pair(nc.gpsimd, 10)
            at_pair(nc.gpsimd, 11)
            nc.gpsimd.dma_start(out=c_t[:, 32:64, :], in_=c[:, 32:64, :])
            at_pair(nc.gpsimd, 12)
            at_pair(nc.gpsimd, 13)
            nc.gpsimd.dma_start(out=at_t[:, 28:30], in_=at[:, 28:30])
            a_group(nc.gpsimd, 13)
            a_group(nc.gpsimd, 14)
            a_group(nc.gpsimd, 15)

            psum_r0 = pr_pool.tile([128, 512], F32)
            psum_r1 = pr_pool.tile([128, 512], F32)
            psum_r = [psum_r0, psum_r1]
            land = tmp_pool.tile([128, 1], F32)
            # Wake the Activation engine early: its first instruction carries
            # a ~1.3us act-table load in the model; pay it off the critical
            # path so the tail copies run at steady-state rate. Source tile
            # is memset by Pool (idle) to avoid waiting on any DMA.
            wsrc = tmp_pool.tile([128, 1], F32)
            warm = tmp_pool.tile([128, 1], F32)
            nc.gpsimd.memset(wsrc[:], 0.0)
            nc.scalar.copy(warm[:], wsrc[:])

            def emit_g2(ch, cp):
                for jj in range(2):
                    nc.tensor.matmul(
                        psum_r[jj][:],
                        cp[:],
                        at_t[:, ch, :, jj * 512 : (jj + 1) * 512],
                        start=(ch == 0),
                        stop=(ch == MCHUNKS - 1),
                        perf_mode=DR,
                    )

            # Process 4 n-chunks per iteration: the G1s for all four land in
            # one 2-bank psum tile, and the update runs as ONE fused
            # STT/TT pair over [128, 4, M] (strided psum AP) — a quarter of
            # the per-instruction DVE overhead of chunk-at-a-time updates.
            for k0 in range(0, NCHUNKS, 4):
                if k0 % 32 == 0:
                    # DVE absorbs this c-half's DMA wait so STT(k0) keeps
                    # only its PSUM-read wait (dedup on the same lane sem).
                    nc.vector.tensor_copy(land[:], c_t[:, k0, 0:1])

                cp = cp_pool.tile([128, 4, M], FP8)
                psum_w = pw_pool.tile([128, 4, 2 * M], F32)
                for dk in range(4):
                    k = k0 + dk
                    j, ub = k // 8, k % 8
                    for q in range(4):
                        nc.tensor.matmul(
                            psum_w[:, dk, :],
                            a_t[:, j, ub, 2 * q : 2 * q + 2, :],
                            ed_t[:, 2 * q : 2 * q + 2, :],
                            start=(q == 0),
                            stop=(q == 3),
                            perf_mode=DR,
                        )

                # psum_w[dk] = [-We | Wa/2];  C'/2 = (1 - We)*(C/2) + Wa/2
                tmp2 = tmp_pool.tile([128, 4, M], F32)
                nc.vector.scalar_tensor_tensor(
                    tmp2[:], psum_w[:, :, 0:M], 1.0, c_t[:, k0 : k0 + 4, :], ADD, MULT
                )
                nc.vector.tensor_add(cp[:], tmp2[:], psum_w[:, :, M : 2 * M])

                emit_g2(k0 // 2, cp[:, 0:2, :])
                emit_g2(k0 // 2 + 1, cp[:, 2:4, :])

            # Split the tail: psum_r bank jj completes at G2(ch=31, jj), so
            # copy+store each half as soon as its accumulation stops. jj0
            # copies on DVE (idle by now) in parallel with jj1 on Act
            # (warmed); stores spread over Pool and SP so neither queues
            # behind the other's issue overhead.
            rt_t0 = rt_pool.tile([128, 512], BF16)
            rt_t1 = rt_pool.tile([128, 512], BF16)
            nc.vector.tensor_copy(rt_t0[:], psum_r0[:])
            s0 = nc.gpsimd.dma_start(out=rt[:, 0:512], in_=rt_t0[:])
            nc.scalar.copy(rt_t1[:], psum_r1[:])
            s1 = nc.sync.dma_start(out=rt[:, 512:1024], in_=rt_t1[:])
            store_names = {s0.ins.name, s1.ins.name}

    # The scheduler can hoist a G1 start-Matmult ahead of the G2 Ldweights
    # whose DVE wait would dedup-cover its bank-WAR wait, leaving it with
    # two waits (PE self-wait + DVE) — one over the HW wait-slot limit.
    # The same-engine self-wait is always satisfied by in-order queue
    # completion, so drop it.
    # The rt stores' RAW wait (on the tail Act copy) transitively follows
    # every input DMA completing, so a DMA-lane credit wait on them is
    # always already satisfied — drop it to stay within the 1-wait limit.
    for inst in nc.inst_map.values():
        si = inst.sync_info
        if si and si.on_wait and len(si.on_wait) > 1:
            eng = str(inst.engine).split(".")[-1]
            kept = [w for w in si.on_wait if not w.ant_name.startswith(eng + "_")]
            if len(kept) > 1 and inst.name in store_names:
                kept = [w for w in kept if not w.ant_name.startswith("DMA")]
            assert len(kept) == 1, (inst.name, [w.ant_name for w in si.on_wait])
            si.on_wait = kept

    return nc


def _prep_inputs(address, erase, add, content):
    f8 = ml_dtypes.float8_e4m3
    a_f8 = address.astype(f8)                                 # [1024, 65536]
    ed = np.concatenate([-erase, 0.5 * add], axis=1)          # [1024, 256] f32
    ed_r = np.ascontiguousarray(
        ed.astype(f8).reshape(BCHUNKS, 128, 2 * M).transpose(1, 0, 2)
    )                                                         # [128, 8, 256]
    c_f8 = (0.5 * content).astype(f8)                         # [65536, 128]

    in_maps = []
    for ci in range(NCORES):
        a_c = a_f8[:, ci * NS : (ci + 1) * NS]                # [1024, 8192]
        # a_r[p, j, ub, bc, u] = A[bc*128+p, j*1024+ub*128+u]
        a_r = np.ascontiguousarray(
            a_c.reshape(BCHUNKS, 128, NSTAGES, 8, 128).transpose(1, 2, 3, 0, 4)
        )                                                     # [128, 8, 8, 8, 128]
        # at_r[p, ch, s, b] = A[b, ch*256 + s*128 + p]
        at_r = np.ascontiguousarray(
            a_c.T.reshape(MCHUNKS, 2, 128, B).transpose(2, 0, 1, 3)
        )                                                     # [128, 32, 2, 1024]
        c_c = c_f8[ci * NS : (ci + 1) * NS, :]
        c_r = np.ascontiguousarray(
            c_c.reshape(NCHUNKS, 128, M).transpose(1, 0, 2)
        )                                                     # [128, 64, 128]
        in_maps.append({"a": a_r, "at": at_r, "c": c_r, "ed": ed_r})
    return in_maps


def kernel(address, erase, add, content, _trace=False, _result_box=None):
    if "nc" not in _compiled:
        _compiled["nc"] = _build_nc()
    nc = _compiled["nc"]

    in_maps = _prep_inputs(address, erase, add, content)
    res = run_bass_kernel_spmd(
        nc, in_maps, core_ids=list(range(NCORES)), trace=_trace
    )
    if _result_box is not None:
        _result_box.append(res)

    acc = np.zeros((M, B), dtype=np.float32)
    for r in res.results:
        acc += np.asarray(r["rt"], dtype=np.float32)
    return np.ascontiguousarray((2.0 * acc).T)


# revision 14
# speedup vs baseline: 2.0252x; 1.0713x over previous
import sys

sys.path.insert(0, "/opt/trn_rl_repo")

import numpy as np
import ml_dtypes

import concourse.mybir as mybir
from concourse import bass, tile
from concourse import tile_sem_assignment as _tsa
from concourse.bass_utils import run_bass_kernel_spmd
from concourse.vector_clock import ScopedClock, VectorClock

_orig_drain_and_barrier = tile.TileContext._drain_and_barrier


def _split_drain_and_barrier(self, tick_clock, wait_clock):
    # The final Drain waits on every active semaphore at once; with 8 HWDGE
    # lanes + SWDGE + 3 engines that exceeds the CTRL instruction's sync
    # wait slots. Emit one 1-wait drain per proc instead (same semantics:
    # SP executes them in order, so all sems reach their targets before the
    # barrier), then replicate the original barrier/cleanup sequence.
    gc = tick_clock.global_clock
    n = _tsa.N_PROCS
    for p in range(n):
        if gc[p] > 0:
            partial = VectorClock([gc[q] if q == p else 0 for q in range(n)])
            d = self.nc.sync.drain()
            wait_clock.add_sem_waits(d.ins, ScopedClock({None: partial}))
    self.nc.all_engine_barrier()
    popped = self.nc._tile_sem_poison_stack.pop()
    assert popped is self._sem_poison
    self.nc.clear_and_free_semaphores(list(self.sems.allocated().values()))
    self.nc.all_engine_barrier()


tile.TileContext._drain_and_barrier = _split_drain_and_barrier

B = 1024        # batch rows of address
N = 65536       # mem rows (sharded)
M = 128         # mem cols
NCORES = 8
NS = N // NCORES          # 8192 rows per core
NCHUNKS = NS // 128       # 64 chunks of 128 mem-rows
MCHUNKS = NS // 256       # 32 mega-chunks of 256 mem-rows (DoubleRow)
BCHUNKS = B // 128        # 8 chunks of 128 batch-rows
NSTAGES = 8               # DMA pipeline stages (8 chunks each)

FP8 = mybir.dt.float8e4
BF16 = mybir.dt.bfloat16
F32 = mybir.dt.float32
DR = mybir.MatmulPerfMode.DoubleRow
ADD = mybir.AluOpType.add
MULT = mybir.AluOpType.mult

_compiled = {}


NG = 16  # a DMA groups; each covers 4 mem-chunks (k) = 2 mega-chunks (ch)


def _build_nc():
    nc = bass.Bass(target_bir_lowering=False)

    # a:  [p=b%128, j(n-slice of 1024), ub(u-block), bc, u]  A shard for GEMM1
    a = nc.dram_tensor("a", [128, NSTAGES, 8, BCHUNKS, 128], FP8, kind="ExternalInput")
    # at: [p=n%128 within 256-chunk, ch, sub, b]  A^T shard for GEMM2 (partition=n)
    at = nc.dram_tensor("at", [128, MCHUNKS, 2, B], FP8, kind="ExternalInput")
    # c:  [p=n%128, k, m]  0.5*content shard (partition=n)
    c = nc.dram_tensor("c", [128, NCHUNKS, M], FP8, kind="ExternalInput")
    # ed: [p=b%128, bc, 2M]  [-erase | 0.5*add] fp8 (hi only)
    ed = nc.dram_tensor("ed", [128, BCHUNKS, 2 * M], FP8, kind="ExternalInput")
    # rt: [m, b] partial (read/2)^T bf16
    rt = nc.dram_tensor("rt", [M, B], BF16, kind="ExternalOutput")

    with tile.TileContext(nc) as tc:
        with (
            tc.tile_pool(name="abuf", bufs=1) as a_pool,
            tc.tile_pool(name="atbuf", bufs=1) as at_pool,
            tc.tile_pool(name="cbuf", bufs=1) as c_pool,
            tc.tile_pool(name="edbuf", bufs=1) as ed_pool,
            tc.tile_pool(name="tmpbuf", bufs=8) as tmp_pool,
            tc.tile_pool(name="cpbuf", bufs=6) as cp_pool,
            tc.tile_pool(name="rtbuf", bufs=2) as rt_pool,
            tc.tile_pool(name="pw", bufs=3, space="PSUM") as pw_pool,
            tc.tile_pool(name="pr", bufs=1, space="PSUM") as pr_pool,
        ):
            a_t = a_pool.tile([128, NSTAGES, 8, BCHUNKS, 128], FP8)
            at_t = at_pool.tile([128, MCHUNKS, 2, B], FP8)
            c_t = c_pool.tile([128, NCHUNKS, M], FP8)
            ed_t = ed_pool.tile([128, BCHUNKS, 2 * M], FP8)

            # Transfers issued from different queues run concurrently in the
            # model (the engine SEQ is the serial resource, ~1.58us per
            # 512KB DMA), so spread the input DMAs across all three DMA
            # queues (SP + Act HWDGE, Pool SWDGE) round-robin in global
            # consumption order: DVE eats one n-chunk per ~330ns and needs
            # the matching a-group and at-pair at the same cadence, with a
            # c quarter every 16 chunks. Preload DMAs write each SBUF dest
            # exactly once, so their only wait is the DGE lane-credit wait.
            pieces = []

            def a_group(g):
                j, ub0 = g // 2, (g % 2) * 4
                pieces.append((a_t[:, j, ub0 : ub0 + 4], a[:, j, ub0 : ub0 + 4]))

            def at_pair(p):
                pieces.append((at_t[:, 2 * p : 2 * p + 2], at[:, 2 * p : 2 * p + 2]))

            def c_quarter(qi):
                pieces.append(
                    (c_t[:, 16 * qi : 16 * qi + 16, :], c[:, 16 * qi : 16 * qi + 16, :])
                )

            pieces.append((ed_t[:], ed[:]))
            a_group(0)
            c_quarter(0)
            at_pair(0)
            for g in range(1, 16):
                a_group(g)
                if g == 3:
                    c_quarter(1)
                if g == 7:
                    c_quarter(2)
                if g == 11:
                    c_quarter(3)
                if g < 15:
                    at_pair(g)
            pieces.append((at_t[:, 30:31], at[:, 30:31]))
            pieces.append((at_t[:, 31:32, :, 0:512], at[:, 31:32, :, 0:512]))
            pieces.append((at_t[:, 31:32, :, 512:1024], at[:, 31:32, :, 512:1024]))

            queues = [nc.sync, nc.scalar, nc.gpsimd]
            for i, (dst, src) in enumerate(pieces):
                queues[i % 3].dma_start(out=dst, in_=src)

            psum_r0 = pr_pool.tile([128, 512], F32)
            psum_r1 = pr_pool.tile([128, 512], F32)
            psum_r = [psum_r0, psum_r1]
            land = tmp_pool.tile([128, 1], F32)
            # Wake the Activation engine early: its first instruction carries
            # a ~1.3us act-table load in the model; pay it off the critical
            # path so the tail copies run at steady-state rate. Source tile
            # is memset by Pool (idle) to avoid waiting on any DMA.
            wsrc = tmp_pool.tile([128, 1], F32)
            warm = tmp_pool.tile([128, 1], F32)
            nc.gpsimd.memset(wsrc[:], 0.0)
            nc.scalar.copy(warm[:], wsrc[:])

            def emit_g2(ch, cp):
                for jj in range(2):
                    nc.tensor.matmul(
                        psum_r[jj][:],
                        cp[:],
                        at_t[:, ch, :, jj * 512 : (jj + 1) * 512],
                        start=(ch == 0),
                        stop=(ch == MCHUNKS - 1),
                        perf_mode=DR,
                    )

            # Process 4 n-chunks per iteration: the G1s for all four land in
            # one 2-bank psum tile, and the update runs as ONE fused
            # STT/TT pair over [128, 4, M] (strided psum AP) — a quarter of
            # the per-instruction DVE overhead of chunk-at-a-time updates.
            for k0 in range(0, NCHUNKS, 4):
                if k0 % 16 == 0:
                    # DVE absorbs this c-quarter's DMA wait so STT(k0) keeps
                    # only its PSUM-read wait (dedup on the same lane sem).
                    nc.vector.tensor_copy(land[:], c_t[:, k0, 0:1])

                cp = cp_pool.tile([128, 4, M], FP8)
                psum_w = pw_pool.tile([128, 4, 2 * M], F32)
                for dk in range(4):
                    k = k0 + dk
                    j, ub = k // 8, k % 8
                    for q in range(4):
                        nc.tensor.matmul(
                            psum_w[:, dk, :],
                            a_t[:, j, ub, 2 * q : 2 * q + 2, :],
                            ed_t[:, 2 * q : 2 * q + 2, :],
                            start=(q == 0),
                            stop=(q == 3),
                            perf_mode=DR,
                        )

                # psum_w[dk] = [-We | Wa/2];  C'/2 = (1 - We)*(C/2) + Wa/2
                tmp2 = tmp_pool.tile([128, 4, M], F32)
                nc.vector.scalar_tensor_tensor(
                    tmp2[:], psum_w[:, :, 0:M], 1.0, c_t[:, k0 : k0 + 4, :], ADD, MULT
                )
                nc.vector.tensor_add(cp[:], tmp2[:], psum_w[:, :, M : 2 * M])

                emit_g2(k0 // 2, cp[:, 0:2, :])
                emit_g2(k0 // 2 + 1, cp[:, 2:4, :])

            # Split the tail: psum_r bank jj completes at G2(ch=31, jj), so
            # copy+store each half as soon as its accumulation stops. jj0
            # copies on DVE (idle by now) in parallel with jj1 on Act
            # (warmed); stores spread over Pool and SP so neither queues
            # behind the other's issue overhead.
            rt_t0 = rt_pool.tile([128, 512], BF16)
            rt_t1 = rt_pool.tile([128, 512], BF16)
            nc.vector.tensor_copy(rt_t0[:], psum_r0[:])
            s0 = nc.gpsimd.dma_start(out=rt[:, 0:512], in_=rt_t0[:])
            nc.scalar.copy(rt_t1[:], psum_r1[:])
            s1 = nc.sync.dma_start(out=rt[:, 512:1024], in_=rt_t1[:])
            store_names = {s0.ins.name, s1.ins.name}

    # The scheduler can hoist a G1 start-Matmult ahead of the G2 Ldweights
    # whose DVE wait would dedup-cover its bank-WAR wait, leaving it with
    # two waits (PE self-wait + DVE) — one over the HW wait-slot limit.
    # The same-engine self-wait is always satisfied by in-order queue
    # completion, so drop it.
    # The rt stores' RAW wait (on the tail Act copy) transitively follows
    # every input DMA completing, so a DMA-lane credit wait on them is
    # always already satisfied — drop it to stay within the 1-wait limit.
    for inst in nc.inst_map.values():
        si = inst.sync_info
        if si and si.on_wait and len(si.on_wait) > 1:
            eng = str(inst.engine).split(".")[-1]
            kept = [w for w in si.on_wait if not w.ant_name.startswith(eng + "_")]
            if len(kept) > 1 and inst.name in store_names:
                kept = [w for w in kept if not w.ant_name.startswith("DMA")]
            assert len(kept) == 1, (inst.name, [w.ant_name for w in si.on_wait])
            si.on_wait = kept

    return nc


def _prep_inputs(address, erase, add, content):
    f8 = ml_dtypes.float8_e4m3
    a_f8 = address.astype(f8)                                 # [1024, 65536]
    ed = np.concatenate([-erase, 0.5 * add], axis=1)          # [1024, 256] f32
    ed_r = np.ascontiguousarray(
        ed.astype(f8).reshape(BCHUNKS, 128, 2 * M).transpose(1, 0, 2)
    )                                                         # [128, 8, 256]
    c_f8 = (0.5 * content).astype(f8)                         # [65536, 128]

    in_maps = []
    for ci in range(NCORES):
        a_c = a_f8[:, ci * NS : (ci + 1) * NS]                # [1024, 8192]
        # a_r[p, j, ub, bc, u] = A[bc*128+p, j*1024+ub*128+u]
        a_r = np.ascontiguousarray(
            a_c.reshape(BCHUNKS, 128, NSTAGES, 8, 128).transpose(1, 2, 3, 0, 4)
        )                                                     # [128, 8, 8, 8, 128]
        # at_r[p, ch, s, b] = A[b, ch*256 + s*128 + p]
        at_r = np.ascontiguousarray(
            a_c.T.reshape(MCHUNKS, 2, 128, B).transpose(2, 0, 1, 3)
        )                                                     # [128, 32, 2, 1024]
        c_c = c_f8[ci * NS : (ci + 1) * NS, :]
        c_r = np.ascontiguousarray(
            c_c.reshape(NCHUNKS, 128, M).transpose(1, 0, 2)
        )                                                     # [128, 64, 128]
        in_maps.append({"a": a_r, "at": at_r, "c": c_r, "ed": ed_r})
    return in_maps


def kernel(address, erase, add, content, _trace=False, _result_box=None):
    if "nc" not in _compiled:
        _compiled["nc"] = _build_nc()
    nc = _compiled["nc"]

    in_maps = _prep_inputs(address, erase, add, content)
    res = run_bass_kernel_spmd(
        nc, in_maps, core_ids=list(range(NCORES)), trace=_trace
    )
    if _result_box is not None:
        _result_box.append(res)

    acc = np.zeros((M, B), dtype=np.float32)
    for r in res.results:
        acc += np.asarray(r["rt"], dtype=np.float32)
    return np.ascontiguousarray((2.0 * acc).T)
